# revision 39
# baseline (speedup 1.0000x reference)
"""Cross-attention block kernel for Trainium2 (8 NeuronCores, data-parallel).

Computes, for full inputs:
    Q = x @ Wq + bq            [B, HW, D]
    K = a @ Wk + bk            [B, S, D]
    V = a @ Wv + bv            [B, S, D]
    out = softmax(Q K^T / sqrt(D)) @ V

Sharding: batch (B=16) split across 8 cores, 2 batches per core. Weights
replicated. No collectives needed.

Per-core strategy (fp8 e4m3 DoubleRow for the attention contractions):
  - Host pre-work: x/audio/weights cast to bf16; Wq,Wk,Wv,bk,bv scaled by 4
    so the fp8 operands use the e4m3 normal range.
  - x and audio transposed to d-major via the DMA XBAR (dma_start_transpose,
    16x128 tiles) -- the PE does no transposes at all.
  - Projections run in bf16 (1 cycle/row, full PE rate): qT = Wq^T @ xT,
    kT = Wk^T @ aT, v = aT^T @ Wv (+bv via a K=1 ones-row matmul folded into
    the same PSUM group).
  - Q-bias folding: softmax(Q K^T) is invariant to per-query shifts, so
    Q stays raw and the per-key correction delta[s] = bq . K'[s] (K' = K+bk)
    is added via the ACT bias port at the exp stage. delta comes from tiny
    DoubleRow matmuls (ap_size=1) against an e4m3 copy of bq.
  - scores^T[s, hw] = kh^T qh in PLAIN e4m3 DoubleRow (2 k-tiles per
    instruction, 0.5 cycles/row -> 4x fp32r rate). Q/K quantization error is
    damped by the softmax scale (std of scaled scores ~0.33), total ~1%.
  - ex = exp(scale*scores + scale*delta + ln8) computed by ACT straight from
    PSUM, split into an e4m3 hi/lo pair (exh = DVE quantize, exl = DVE sub).
    The ln8 bias scales ex by 8 so the whole range [1.3, 48] is normal e4m3.
  - out = (exh+exl) @ (vh+vl) with the lo*lo term dropped: hi*hi uses
    k-tile-paired DoubleRow; the cross terms exl*vh and exh*vl share single
    DoubleRow instructions (slot0=lo*hi, slot1=hi*lo). 1.33x fp32r rate.
  - denominator: DoubleRow against a constant 4.0 column (matching the 4x
    pre-scale of V), interleaved into a second PSUM bank; the final ACT copy
    applies the reciprocal as a per-partition scale.
  - The out stage is software-pipelined one block behind scores/exp so the
    ACT/DVE queues never stall the PE.
"""

from contextlib import ExitStack

import ml_dtypes
import numpy as np

import concourse.bass as bass
import concourse.bacc as bacc
import concourse.mybir as mybir
import concourse.tile as tile
from concourse.bass_utils import run_bass_kernel_spmd

P = 128
D = 512          # d_query == d_audio == d_out
CD = D // P      # 4 chunks of the feature dim
HW = 4096        # queries per batch
S = 1024         # keys per batch
SC = S // P      # 8 s-chunks
HWB = 512        # hw rows processed per block
NBLK = HW // HWB
B_FULL = 16
N_CORES = 8
BL = B_FULL // N_CORES  # 2 batches per core
SCALE = 1.0 / float(np.sqrt(D))
LN8 = float(np.log(8.0))

f32 = mybir.dt.float32
bf16 = mybir.dt.bfloat16
e4 = mybir.dt.float8e4
AFT = mybir.ActivationFunctionType
ALU = mybir.AluOpType
DR = mybir.MatmulPerfMode.DoubleRow

BF16NP = ml_dtypes.bfloat16


def build_nc():
    nc = bacc.Bacc("TRN2", target_bir_lowering=False, debug=False)

    # x/audio arrive HOST-TRANSPOSED (d-major) and HOST-SPLIT into e4m3
    # hi/lo pairs: dim0 of the pair axis is hi for data, lo for weights
    x = nc.dram_tensor("x", [BL, D, HW], e4, kind="ExternalInput").ap()
    audio = nc.dram_tensor("audio_embed", [BL, 2, D, S], e4, kind="ExternalInput").ap()
    wq = nc.dram_tensor("Wq", [2, D, D], e4, kind="ExternalInput").ap()
    bq = nc.dram_tensor("bq", [D], f32, kind="ExternalInput").ap()
    wk = nc.dram_tensor("Wk", [2, D, D], e4, kind="ExternalInput").ap()
    bk = nc.dram_tensor("bk", [D], bf16, kind="ExternalInput").ap()
    wv = nc.dram_tensor("Wv", [2, D, D], e4, kind="ExternalInput").ap()
    bv = nc.dram_tensor("bv", [D], bf16, kind="ExternalInput").ap()
    out = nc.dram_tensor("out", [BL, HW, D], bf16, kind="ExternalOutput").ap()

    with tile.TileContext(nc) as tc:
        with ExitStack() as ctx:
            _body(ctx, tc, x, audio, wq, bq, wk, bk, wv, bv, out)

    nc.compile()
    return nc


def _body(ctx, tc, x, audio, wq, bq, wk, bk, wv, bv, out):
    nc = tc.nc

    const_pool = ctx.enter_context(tc.tile_pool(name="const", bufs=1))
    batch_pool = ctx.enter_context(tc.tile_pool(name="batch", bufs=2))
    work_pool = ctx.enter_context(tc.tile_pool(name="work", bufs=2))
    small_pool = ctx.enter_context(tc.tile_pool(name="small", bufs=4))
    psum_mm = ctx.enter_context(tc.tile_pool(name="pmm", bufs=4, space="PSUM"))
    psum_sc = ctx.enter_context(tc.tile_pool(name="psc", bufs=2, space="PSUM"))
    psum_den = ctx.enter_context(tc.tile_pool(name="pden", bufs=1, space="PSUM"))
    psum_dl = ctx.enter_context(tc.tile_pool(name="pdl", bufs=1, space="PSUM"))

    # Weight/bias loads are interleaved with the first audio chunks so the
    # first transposable input data leads the serial DMA queue.
    consts = {}

    def _load_small_consts():
        bk_row = const_pool.tile([1, D], bf16)
        nc.sync.dma_start(bk_row, bk[None, :])
        bq_f = const_pool.tile([P, CD], f32)
        nc.sync.dma_start(bq_f, bq.rearrange("(c p) -> p c", p=P))
        bq8 = const_pool.tile([P, CD], e4)
        nc.vector.tensor_copy(bq8, bq_f)
        bv_row = const_pool.tile([1, D], bf16)
        nc.sync.dma_start(bv_row, bv[None, :])
        ones_row = const_pool.tile([1, P], bf16)
        nc.gpsimd.memset(ones_row, 1.0)
        ones512 = const_pool.tile([1, 512], bf16)
        nc.gpsimd.memset(ones512, 1.0)
        fours = const_pool.tile([P, 2, 1], e4)
        nc.gpsimd.memset(fours, 16.0)
        consts.update(bk_row=bk_row, bq8=bq8, bv_row=bv_row,
                      ones_row=ones_row, ones512=ones512, fours=fours)

    def _load_w(name, t, queue=None):
        # [P, 2, CD, D] e4m3: [:,0]=lo, [:,1]=hi
        w_sb = const_pool.tile([P, 2, CD, D], e4, name=f"w_sb_{name}")
        (queue or nc.sync).dma_start(
            w_sb, t.rearrange("a (c p) n -> p a c n", p=P)
        )
        consts[name] = w_sb

    def emit_audio_loads(b):
        """audio arrives d-major, e4m3 hi/lo pair: [:,0]=hi, [:,1]=lo."""
        aT = batch_pool.tile([P, 2, CD, S], e4, tag="aT")
        a_view = audio[b].rearrange("a (c p) s -> p a c s", p=P)
        nc.sync.dma_start(aT[:, :, :, 0:512], a_view[:, :, :, 0:512])
        if b == 0:
            _load_w("wk_sb", wk)
        nc.sync.dma_start(aT[:, :, :, 512:1024], a_view[:, :, :, 512:1024])
        if b == 0:
            _load_w("wv_sb", wv)
            _load_small_consts()
        return aT

    def emit_audio_compute(b, aT):
        """K' hi/lo, V hi/lo, delta, and KQ = K'Wq^T (query projection
        folded into the keys: 4096 queries never touch Wq)."""
        klh = batch_pool.tile([P, 2, CD, S], e4, tag="kh")  # [:,0]=hi [:,1]=lo
        kqh = batch_pool.tile([P, CD, S], e4, tag="kq")
        vhl = batch_pool.tile([P, 2, SC, D], e4, tag="v")  # [:,0]=hi [:,1]=lo
        dT_ps = psum_dl.tile([P, SC], f32, tag="dl")
        dsb = batch_pool.tile([P, SC], f32, tag="dsb")
        for half in range(2):
            hsl = slice(half * 512, (half + 1) * 512)
            for m in range(CD):
                mm_ps = psum_mm.tile([P, 512], f32, tag="mm")
                ms = slice(m * P, (m + 1) * P)
                for t in range(2):
                    nc.tensor.matmul(
                        mm_ps,
                        consts["wk_sb"][:, 1, 2 * t : 2 * t + 2, ms],
                        aT[:, 0, 2 * t : 2 * t + 2, hsl],
                        start=(t == 0),
                        stop=False,
                        perf_mode=DR,
                    )
                for t in range(CD):
                    nc.tensor.matmul(
                        mm_ps,
                        consts["wk_sb"][:, :, t, ms],
                        aT[:, :, t, hsl],
                        start=False,
                        stop=False,
                        perf_mode=DR,
                    )
                # bk fold: psum += bk_chunk^T (x) ones
                nc.tensor.matmul(
                    mm_ps, consts["bk_row"][:, ms], consts["ones512"],
                    start=False, stop=True,
                )
                nc.scalar.activation(klh[:, 0, m, hsl], mm_ps, AFT.Copy)
                nc.vector.tensor_tensor(
                    klh[:, 1, m, hsl], mm_ps, klh[:, 0, m, hsl], ALU.subtract
                )
            for g in range(half * 4, half * 4 + 4):
                mm_ps = psum_mm.tile([P, D], f32, tag="mm")
                gs = slice(g * P, (g + 1) * P)
                for t in range(2):
                    nc.tensor.matmul(
                        mm_ps,
                        aT[:, 0, 2 * t : 2 * t + 2, gs],
                        consts["wv_sb"][:, 1, 2 * t : 2 * t + 2, :],
                        start=(t == 0),
                        stop=False,
                        perf_mode=DR,
                    )
                for t in range(CD):
                    nc.tensor.matmul(
                        mm_ps,
                        aT[:, :, t, gs],
                        consts["wv_sb"][:, :, t, :],
                        start=False,
                        stop=False,
                        perf_mode=DR,
                    )
                nc.tensor.matmul(
                    mm_ps, consts["ones_row"], consts["bv_row"],
                    start=False, stop=True,
                )
                nc.scalar.activation(vhl[:, 0, g, :], mm_ps, AFT.Copy)
                nc.vector.tensor_tensor(
                    vhl[:, 1, g, :], mm_ps, vhl[:, 0, g, :], ALU.subtract
                )
            # delta[s] = bq . K'[s] for this half's s-chunks (tiny DoubleRow)
            for g in range(half * 4, half * 4 + 4):
                for t in range(2):
                    nc.tensor.matmul(
                        dT_ps[:, g : g + 1],
                        klh[:, 0, 2 * t : 2 * t + 2, g * P : (g + 1) * P],
                        consts["bq8"][:, 2 * t : 2 * t + 2, None],
                        start=(t == 0),
                        stop=(t == 1),
                        perf_mode=DR,
                    )
            # KQ^T[d_in, s] = Wq^T-pair . K'-pair for this half
            for m in range(CD):
                kq_ps = psum_mm.tile([P, 512], f32, tag="mm")
                ms = slice(m * P, (m + 1) * P)
                for t in range(2):
                    nc.tensor.matmul(
                        kq_ps,
                        consts["wq_sb"][:, 1, 2 * t : 2 * t + 2, ms],
                        klh[:, 0, 2 * t : 2 * t + 2, hsl],
                        start=(t == 0),
                        stop=False,
                        perf_mode=DR,
                    )
                for t in range(CD):
                    nc.tensor.matmul(
                        kq_ps,
                        consts["wq_sb"][:, :, t, ms],
                        klh[:, :, t, hsl],
                        start=False,
                        stop=(t == CD - 1),
                        perf_mode=DR,
                    )
                nc.vector.tensor_scalar(
                    kqh[:, m, hsl], kq_ps, 1.0 / 16.0, None, ALU.mult
                )
        # dsb = (SCALE/256) * dT + ln(8): exp-stage per-partition bias
        nc.vector.tensor_scalar(dsb, dT_ps, SCALE / 256.0, LN8, ALU.mult, ALU.add)
        return {"kqh": kqh, "vhl": vhl, "dsb": dsb}

    def emit_x_loads(b, blk):
        """x arrives d-major, e4m3 hi only."""
        xT = work_pool.tile([P, CD, HWB], e4, tag="xT", bufs=4)
        nc.sync.dma_start(
            xT,
            x[b].rearrange("(c p) w -> p c w", p=P)[
                :, :, blk * HWB : (blk + 1) * HWB
            ],
        )
        return xT

    def emit_scores_stage(bst, st):
        kqh, dsb = bst["kqh"], bst["dsb"]
        xh = st.pop("xh")
        exlh = work_pool.tile([P, 2, SC, HWB], e4, tag="ex")  # [:,0]=lo [:,1]=hi
        for g in range(SC):
            sc_ps = psum_sc.tile([P, HWB], f32, tag="sc")
            for t in range(2):
                nc.tensor.matmul(
                    sc_ps,
                    kqh[:, 2 * t : 2 * t + 2, g * P : (g + 1) * P],
                    xh[:, 2 * t : 2 * t + 2, :],
                    start=(t == 0),
                    stop=(t == 1),
                    perf_mode=DR,
                )
            ex_f = small_pool.tile([P, HWB], f32, tag="exf", bufs=8)
            nc.scalar.activation(
                ex_f, sc_ps, AFT.Exp, bias=dsb[:, g, None], scale=SCALE / 16.0
            )
            eng = nc.gpsimd if g % 2 == 0 else nc.vector
            eng.tensor_copy(exlh[:, 1, g, :], ex_f)
            nc.vector.tensor_tensor(
                exlh[:, 0, g, :], ex_f, exlh[:, 1, g, :], ALU.subtract
            )
        st["exlh"] = exlh

    def emit_out_stage(bst, st, b, blk, last=False):
        exlh = st.pop("exlh")
        vhl = bst["vhl"]
        out_view = out[b].rearrange("(t h p) n -> t p h n", p=P, h=CD)[blk]
        out_sb = work_pool.tile([P, CD, D], bf16, tag="o")
        den_all = psum_den.tile([P, CD], f32, tag="den")
        for h in range(CD):
            hs = slice(h * P, (h + 1) * P)
            num_ps = psum_mm.tile([P, D], f32, tag="mm")
            den_ps = den_all[:, h : h + 1]
            # hi*hi first (needs only exh), then den (so the reciprocal
            # overlaps the cross matmuls), then the cross terms
            for t in range(SC // 2):
                nc.tensor.matmul(
                    num_ps,
                    exlh[:, 1, 2 * t : 2 * t + 2, hs],
                    vhl[:, 0, 2 * t : 2 * t + 2, :],
                    start=(t == 0),
                    stop=False,
                    perf_mode=DR,
                )
            # cross terms (slot0 = exl*vh, slot1 = exh*vl) with den
            # interleaved: den(t) shares exl(t) readiness with cross(t)
            for t in range(SC):
                nc.tensor.matmul(
                    num_ps,
                    exlh[:, :, t, hs],
                    vhl[:, :, t, :],
                    start=False,
                    stop=(t == SC - 1),
                    perf_mode=DR,
                )
                nc.tensor.matmul(
                    den_ps,
                    exlh[:, :, t, hs],
                    consts["fours"],
                    start=(t == 0),
                    stop=(t == SC - 1),
                    perf_mode=DR,
                )
            rec = small_pool.tile([P, 1], f32, tag="rec")
            nc.vector.reciprocal(rec, den_ps)
            osc_dve = h % 2 == 1 and not last
            if osc_dve:
                nc.vector.tensor_scalar(
                    out_sb[:, h, :], num_ps, rec, None, ALU.mult
                )
            else:
                nc.scalar.activation(
                    out_sb[:, h, :], num_ps, AFT.Copy, bias=0.0, scale=rec
                )
            if last:
                nc.scalar.dma_start(out_view[:, h, :], out_sb[:, h, :])
        # store on the ACT hwdge queue (so x loads/XBARs never queue behind it)
        if not last:
            nc.scalar.dma_start(out_view, out_sb)

    # --- staged global loop: x loads LEAD steps ahead, qT one block ahead
    # of scores, out one block behind ------------------------------------
    TOT = BL * NBLK
    LEAD = 2
    AUDIO_TRIGGER = 5  # prefetch batch b+1's audio loads at blk 5 of batch b
    bstates = {}
    stages = {}
    aT_pend = {}
    xT_pend = {}
    for s in range(TOT + 1):
        if s < TOT:
            b, blk = divmod(s, NBLK)
            if s == 0:
                # PE warm-up: dummy matmuls ramp the tensor-engine p-state
                # to full clock while the startup DMAs land
                warm = const_pool.tile([P, P], bf16)
                nc.gpsimd.memset(warm, 0.0)
                # dummy activation pulls the 1.28us act-table load into the
                # DMA-bound startup window
                act_w0 = const_pool.tile([P, 1], f32)
                nc.gpsimd.memset(act_w0, 0.0)
                act_w1 = const_pool.tile([P, 1], f32)
                nc.scalar.activation(act_w1, act_w0, AFT.Exp)
                warm_ps = psum_mm.tile([P, P], f32, tag="mm")
                for i in range(40):
                    nc.tensor.matmul(
                        warm_ps, warm, warm, start=(i == 0), stop=(i == 39),
                    )
                aT_pend[0] = emit_audio_loads(0)
                _load_w("wq_sb", wq)
                xT_pend[0] = emit_x_loads(0, 0)
                bstates[0] = emit_audio_compute(0, aT_pend.pop(0))
                for k in range(1, LEAD + 2):
                    xT_pend[k] = emit_x_loads(*divmod(k, NBLK))
            if blk == 0 and b > 0:
                bstates[b] = emit_audio_compute(b, aT_pend.pop(b))
            if s + LEAD + 1 < TOT:
                xT_pend[s + LEAD + 1] = emit_x_loads(*divmod(s + LEAD + 1, NBLK))
            st = stages[s] = {}
            st["xh"] = xT_pend.pop(s)
            emit_scores_stage(bstates[divmod(s, NBLK)[0]], stages[s])
            if blk == AUDIO_TRIGGER and b + 1 < BL:
                aT_pend[b + 1] = emit_audio_loads(b + 1)
        if 1 <= s <= TOT:
            b, blk = divmod(s - 1, NBLK)
            emit_out_stage(bstates[b], stages.pop(s - 1), b, blk, last=(s == TOT))


_NC_CACHE = None


def _get_nc():
    global _NC_CACHE
    if _NC_CACHE is None:
        _NC_CACHE = build_nc()
    return _NC_CACHE


E4NP = ml_dtypes.float8_e4m3


def _split8(a, hi_first):
    hi = a.astype(E4NP)
    lo = (a - hi.astype(np.float32)).astype(E4NP)
    pair = [hi, lo] if hi_first else [lo, hi]
    return np.ascontiguousarray(np.stack(pair, axis=-3))


def make_in_maps(inputs):
    """Host-side prep: transpose + e4m3 hi/lo splits, 4x scaling of W/bk/bv."""
    x = np.asarray(inputs["x"], dtype=np.float32)
    audio = np.asarray(inputs["audio_embed"], dtype=np.float32)
    wq = _split8(
        np.ascontiguousarray(np.asarray(inputs["Wq"], dtype=np.float32).T) * 16.0,
        False,
    )
    bq = np.ascontiguousarray(np.asarray(inputs["bq"], dtype=np.float32) * 16.0)
    wk = _split8(np.asarray(inputs["Wk"], dtype=np.float32) * 16.0, False)
    bk = (np.asarray(inputs["bk"], dtype=np.float32) * 16.0).astype(BF16NP)
    wv = _split8(np.asarray(inputs["Wv"], dtype=np.float32) * 16.0, False)
    bv = (np.asarray(inputs["bv"], dtype=np.float32) * 16.0).astype(BF16NP)
    xb = np.ascontiguousarray(x.transpose(0, 2, 1)).astype(E4NP)
    ab = _split8(np.ascontiguousarray(audio.transpose(0, 2, 1)), True)
    in_maps = []
    for i in range(N_CORES):
        in_maps.append(
            {
                "x": np.ascontiguousarray(xb[i * BL : (i + 1) * BL]),
                "audio_embed": np.ascontiguousarray(ab[i * BL : (i + 1) * BL]),
                "Wq": wq,
                "bq": bq,
                "Wk": wk,
                "bk": bk,
                "Wv": wv,
                "bv": bv,
            }
        )
    return in_maps


def kernel(**inputs):
    nc = _get_nc()
    in_maps = make_in_maps(inputs)
    res = run_bass_kernel_spmd(nc, in_maps, core_ids=list(range(N_CORES)))
    return np.concatenate(
        [np.asarray(res.results[i]["out"]) for i in range(N_CORES)], axis=0
    ).astype(np.float32)
